# revision 6
# baseline (speedup 1.0000x reference)
"""Causal multi-head self-attention with RoPE on 8 Trainium2 NeuronCores.

Problem: b=4, s=2048, d_model=1024, 16 heads, dk=64, causal, RoPE(theta=1e4).

Sharding: 8 cores = (batch b, head-half) pairs. Core c handles batch c//2 and
heads (c%2)*8 .. (c%2)*8+8. Each core computes its 8 heads' Q/K/V projections,
causal attention, and a partial output projection; the host sums the two
partials per batch.

Per-core dataflow (all matmuls fp32r, 1 cycle/row on the PE):
  phase 1  Q^T/K^T = (W_perm^T)^T-matmul with moving x^T   -> [128(2 heads), 2048]
           V       = x^T^T-matmul with moving Wv^T         -> [tok, 8*65] ([V|1] interleave)
  phase 2  RoPE in place on Q^T/K^T: pair-swap via SBUF->SBUF DMA + 3 elementwise
  phase 3  per head-pair g, q-chunk c (512): for k-tile kt<=4c+3:
              scores^T[128k, 512q] x2 heads (row-packed K=64 matmuls)
              P = exp(S/8) (one ACT op over both heads), diag-mask multiply
              out^T[65,512] += [V|1]^T @ P  (accumulating PSUM, sums ride row 64)
           staging -> DRAM bounce outT[8, 65, 2048]
  phase 4  reload outT, per-(head,tok) normalize by bcast 1/sums, Wo projection
           y_partial[2048, 1024] -> DRAM

RoPE trick: head-dim pre-permuted (on host, inside W) to [evens; odds] blocks so
rotation = Q*cos + swap32(Q)*sin with pure 32-partition block swaps.
"""
import sys
import numpy as np

for _p in ('/root/.axon_site/_ro/trn_rl_repo', '/opt/trn_rl_repo'):
    if _p not in sys.path:
        sys.path.append(_p)

import concourse.bass as bass
import concourse.tile as tile
from concourse import bacc, mybir
from concourse.bass_utils import run_bass_kernel_spmd

F32 = mybir.dt.float32
F32R = mybir.dt.float32r
EXP = mybir.ActivationFunctionType.Exp

B, S, D = 4, 2048, 1024
NH, DK = 16, 64
NHC = 8            # heads per core
HD = NHC * DK      # 512
NG = 4             # head-pairs (groups) per core
NC_CHUNK = 512     # q-chunk
N_CHUNKS = S // NC_CHUNK       # 4
N_KT = S // 128                # 16
KSUB = D // 128                # 8
THETA = 10000.0

_CACHED = {}


def _build():
    nc = bacc.Bacc('TRN2', target_bir_lowering=False, debug=False, num_devices=8)
    xT = nc.dram_tensor('xT', [D, S], F32R, kind='ExternalInput').ap()
    wqT = nc.dram_tensor('wqT', [D, HD], F32R, kind='ExternalInput').ap()
    wkT = nc.dram_tensor('wkT', [D, HD], F32R, kind='ExternalInput').ap()
    wvT = nc.dram_tensor('wvT', [D, HD], F32R, kind='ExternalInput').ap()
    woT = nc.dram_tensor('woT', [HD, D], F32R, kind='ExternalInput').ap()
    cosd = nc.dram_tensor('cosd', [128, S], F32, kind='ExternalInput').ap()
    sind = nc.dram_tensor('sind', [128, S], F32, kind='ExternalInput').ap()
    maskd = nc.dram_tensor('maskd', [4, 128, 2 * NC_CHUNK], F32, kind='ExternalInput').ap()
    y = nc.dram_tensor('y', [S, D], F32, kind='ExternalOutput').ap()
    outT = nc.dram_tensor('outT', [NHC, DK + 1, S], F32).ap()   # DRAM bounce

    with tile.TileContext(nc) as tc:
        with tc.tile_pool(name='persist', bufs=1) as persist:
            q_sb = persist.tile([128, NG, S], F32R, tag='q_sb')
            k_sb = persist.tile([128, NG, S], F32R, tag='k_sb')
            v_sb = persist.tile([128, N_KT, NHC * (DK + 1)], F32R, tag='v_sb')
            v4 = v_sb.rearrange('p t (h m) -> p t h m', h=NHC)

            # ---------------- phase 1: QKV projections ----------------
            with tc.tile_pool(name='xw', bufs=1) as xw, \
                 tc.tile_pool(name='wpool', bufs=1) as wpool, \
                 tc.tile_pool(name='ps1', bufs=3, space='PSUM') as ps1:
                x_sb = xw.tile([128, KSUB, S], F32R, tag='x_sb')
                for s in range(KSUB):
                    nc.sync.dma_start(x_sb[:, s], xT[128 * s:128 * (s + 1), :])

                # Q^T / K^T: [128 hd (2 heads), tok]
                for name, w_ap, dst in (('q', wqT, q_sb), ('k', wkT, k_sb)):
                    w_sb = wpool.tile([128, KSUB, HD], F32R, tag='w_in')
                    for s in range(KSUB):
                        nc.sync.dma_start(w_sb[:, s], w_ap[128 * s:128 * (s + 1), :])
                    for g in range(NG):
                        for c in range(N_CHUNKS):
                            ps = ps1.tile([128, NC_CHUNK], F32, tag='ps1')
                            for s in range(KSUB):
                                nc.tensor.matmul(
                                    ps[:], w_sb[:, s, 128 * g:128 * (g + 1)],
                                    x_sb[:, s, NC_CHUNK * c:NC_CHUNK * (c + 1)],
                                    start=(s == 0), stop=(s == KSUB - 1))
                            nc.vector.tensor_copy(
                                dst[:, g, NC_CHUNK * c:NC_CHUNK * (c + 1)], ps[:])

                # V natural: [tok, hd] with the [V|1] interleave
                w_sb = wpool.tile([128, KSUB, HD], F32R, tag='w_in')
                for s in range(KSUB):
                    nc.sync.dma_start(w_sb[:, s], wvT[128 * s:128 * (s + 1), :])
                ones_sb = wpool.tile([128, NHC], F32, tag='ones')
                nc.vector.memset(ones_sb[:], 1.0)
                for t in range(N_KT):
                    nc.vector.tensor_copy(v4[:, t, :, DK], ones_sb[:])
                for t in range(N_KT):
                    ps = ps1.tile([128, NC_CHUNK], F32, tag='ps1')
                    for s in range(KSUB):
                        nc.tensor.matmul(
                            ps[:], x_sb[:, s, 128 * t:128 * (t + 1)], w_sb[:, s],
                            start=(s == 0), stop=(s == KSUB - 1))
                    nc.vector.tensor_copy(
                        v4[:, t, :, 0:DK],
                        ps.rearrange('p (h m) -> p h m', h=NHC))

            # ---------------- phase 2: RoPE in place on Q^T/K^T ----------------
            with tc.tile_pool(name='tables', bufs=1) as tables, \
                 tc.tile_pool(name='ropetmp', bufs=3) as ropetmp:
                cos_sb = tables.tile([128, S], F32, tag='cos_sb')
                nc.sync.dma_start(cos_sb[:], cosd)
                sin_sb = tables.tile([128, S], F32, tag='sin_sb')
                nc.sync.dma_start(sin_sb[:], sind)

                for g in range(NG):
                    for t_sb in (q_sb, k_sb):
                        for c in range(N_CHUNKS):
                            cs = slice(NC_CHUNK * c, NC_CHUNK * (c + 1))
                            ch = t_sb[:, g, cs]
                            sw = ropetmp.tile([128, NC_CHUNK], F32R, tag='swap')
                            for blk in range(4):
                                src = (blk // 2) * 64 + (1 - blk % 2) * 32
                                nc.sync.dma_start(sw[32 * blk:32 * (blk + 1), :],
                                                  ch[src:src + 32, :])
                            t1 = ropetmp.tile([128, NC_CHUNK], F32, tag='t1')
                            nc.vector.tensor_mul(t1[:], ch, cos_sb[:, cs])
                            t2 = ropetmp.tile([128, NC_CHUNK], F32, tag='t2')
                            nc.gpsimd.tensor_tensor(t2[:], sw[:], sin_sb[:, cs],
                                                    mybir.AluOpType.mult)
                            nc.vector.tensor_add(ch, t1[:], t2[:])

            # ---------------- phase 3: attention ----------------
            with tc.tile_pool(name='masks', bufs=1) as maskp, \
                 tc.tile_pool(name='pp', bufs=4) as pp, \
                 tc.tile_pool(name='stage', bufs=3) as stage, \
                 tc.tile_pool(name='scps', bufs=2, space='PSUM') as scps, \
                 tc.tile_pool(name='outps', bufs=4, space='PSUM') as outps:
                mask_sb = maskp.tile([128, 4, 2 * NC_CHUNK], F32, tag='mask_sb')
                nc.sync.dma_start(
                    mask_sb[:], bass.AP(
                        tensor=maskd.tensor, offset=maskd.offset,
                        ap=[[2 * NC_CHUNK, 128], [128 * 2 * NC_CHUNK, 4],
                            [1, 2 * NC_CHUNK]]))
                n_mask = 0
                for g in range(NG):
                    hA, hB = 2 * g, 2 * g + 1
                    for c in range(N_CHUNKS):
                        cs = slice(NC_CHUNK * c, NC_CHUNK * (c + 1))
                        oA = outps.tile([DK + 1, NC_CHUNK], F32, tag='outps')
                        oB = outps.tile([DK + 1, NC_CHUNK], F32, tag='outps')
                        n_kt = 4 * (c + 1)
                        for kt in range(n_kt):
                            ks = slice(128 * kt, 128 * (kt + 1))
                            sc = scps.tile([128, 2 * NC_CHUNK], F32, tag='sc')
                            nc.tensor.matmul(sc[:, 0:NC_CHUNK],
                                             k_sb[0:64, g, ks], q_sb[0:64, g, cs],
                                             start=True, stop=True)
                            nc.tensor.matmul(sc[:, NC_CHUNK:],
                                             k_sb[64:128, g, ks], q_sb[64:128, g, cs],
                                             start=True, stop=True)
                            p = pp.tile([128, 2 * NC_CHUNK], F32R, tag='p')
                            nc.scalar.activation(out=p[:], in_=sc[:], func=EXP,
                                                 scale=1.0 / np.sqrt(DK))
                            if kt >= 4 * c:          # diagonal: mask
                                j = kt - 4 * c
                                eng = nc.vector if n_mask % 2 == 0 else nc.gpsimd
                                eng.tensor_tensor(p[:], p[:], mask_sb[:, j],
                                                  mybir.AluOpType.mult)
                                n_mask += 1
                            nc.tensor.matmul(oA[:], v_sb[:, kt, 65 * hA:65 * (hA + 1)],
                                             p[:, 0:NC_CHUNK],
                                             start=(kt == 0), stop=(kt == n_kt - 1))
                            nc.tensor.matmul(oB[:], v_sb[:, kt, 65 * hB:65 * (hB + 1)],
                                             p[:, NC_CHUNK:],
                                             start=(kt == 0), stop=(kt == n_kt - 1))
                        st = stage.tile([DK + 1, 2 * NC_CHUNK], F32, tag='stage')
                        nc.vector.tensor_copy(st[:, 0:NC_CHUNK], oA[:])
                        nc.vector.tensor_copy(st[:, NC_CHUNK:], oB[:])
                        nc.sync.dma_start(outT[hA, :, cs], st[:, 0:NC_CHUNK])
                        nc.sync.dma_start(outT[hB, :, cs], st[:, NC_CHUNK:])

            # ---------------- phase 4: normalize + output projection ----------------
            with tc.tile_pool(name='wo', bufs=1) as wo, \
                 tc.tile_pool(name='lhs', bufs=1) as lhs, \
                 tc.tile_pool(name='otrc', bufs=2) as otrc, \
                 tc.tile_pool(name='ytmp', bufs=4) as ytmp, \
                 tc.tile_pool(name='ps4', bufs=4, space='PSUM') as ps4:
                wo_sb = wo.tile([128, NG, D], F32R, tag='wo_sb')
                for g in range(NG):
                    nc.sync.dma_start(wo_sb[:, g], woT[128 * g:128 * (g + 1), :])
                lhs_sb = lhs.tile([128, NG, S], F32R, tag='lhs_sb')
                for g in range(NG):
                    hA, hB = 2 * g, 2 * g + 1
                    ot = otrc.tile([128, S], F32, tag='ot')
                    nc.sync.dma_start(ot[0:64, :], outT[hA, 0:DK, :])
                    nc.sync.dma_start(ot[64:128, :], outT[hB, 0:DK, :])
                    rc = otrc.tile([128, S], F32, tag='rc')
                    for half, h in ((0, hA), (1, hB)):
                        nc.sync.dma_start(
                            rc[64 * half:64 * (half + 1), :],
                            bass.AP(tensor=outT.tensor,
                                    offset=outT.offset + (h * (DK + 1) + DK) * S,
                                    ap=[[0, 64], [1, S]]))
                    nc.vector.reciprocal(rc[:], rc[:])
                    nc.vector.tensor_mul(lhs_sb[:, g], ot[:], rc[:])

                for t in range(N_KT):
                    ts_ = slice(128 * t, 128 * (t + 1))
                    for half in range(2):
                        ps = ps4.tile([128, 512], F32, tag='ps4')
                        for g in range(NG):
                            nc.tensor.matmul(ps[:], lhs_sb[:, g, ts_],
                                             wo_sb[:, g, 512 * half:512 * (half + 1)],
                                             start=(g == 0), stop=(g == NG - 1))
                        yt = ytmp.tile([128, 512], F32, tag='yt')
                        nc.scalar.copy(yt[:], ps[:])
                        nc.sync.dma_start(y[ts_, 512 * half:512 * (half + 1)], yt[:])
    nc.compile()
    return nc


def _host_inputs(x, Wq, Wk, Wv, Wo, token_positions):
    """Per-core input maps (host-side sharding / layout only)."""
    perm = np.empty(DK, np.int64)
    perm[0:32] = np.arange(0, DK, 2)
    perm[32:64] = np.arange(1, DK, 2)

    inv_freq = 1.0 / (THETA ** (np.arange(0, DK, 2, dtype=np.float64) / DK))  # [32]
    ang = token_positions.astype(np.float64)[None, :] * inv_freq[:, None]     # [32, S]
    cos32 = np.cos(ang).astype(np.float32)
    sin32 = np.sin(ang).astype(np.float32)
    cos128 = np.tile(cos32, (4, 1))
    sin128 = np.concatenate([-sin32, sin32, -sin32, sin32], axis=0)

    qc = np.arange(NC_CHUNK)[None, :]
    kr = np.arange(128)[:, None]
    masks = np.stack([(qc >= 128 * j + kr).astype(np.float32) for j in range(4)])
    masks = np.concatenate([masks, masks], axis=2)  # [4, 128, 1024]

    in_maps = []
    for core in range(8):
        b = core // 2
        h0 = (core % 2) * NHC
        cols = slice(h0 * DK, (h0 + NHC) * DK)
        wq_s = Wq[cols, :].reshape(NHC, DK, D)[:, perm, :].reshape(HD, D)
        wk_s = Wk[cols, :].reshape(NHC, DK, D)[:, perm, :].reshape(HD, D)
        in_maps.append({
            'xT': np.ascontiguousarray(x[b].T),
            'wqT': np.ascontiguousarray(wq_s.T),
            'wkT': np.ascontiguousarray(wk_s.T),
            'wvT': np.ascontiguousarray(Wv[cols, :].T),
            'woT': np.ascontiguousarray(Wo[:, cols].T),
            'cosd': cos128, 'sind': sin128, 'maskd': masks,
        })
    return in_maps


def kernel(x, Wq, Wk, Wv, Wo, token_positions, _results_hook=None):
    if 'nc' not in _CACHED:
        _CACHED['nc'] = _build()
    nc = _CACHED['nc']
    in_maps = _host_inputs(np.asarray(x), np.asarray(Wq), np.asarray(Wk),
                           np.asarray(Wv), np.asarray(Wo),
                           np.asarray(token_positions))
    res = run_bass_kernel_spmd(nc, in_maps, list(range(8)),
                               **(_results_hook or {}))
    if _results_hook is not None:
        _CACHED['last'] = res
    out = np.empty((B, S, D), np.float32)
    for b in range(B):
        out[b] = res.results[2 * b]['y'] + res.results[2 * b + 1]['y']
    return out


# revision 9
# speedup vs baseline: 1.1231x; 1.1231x over previous
"""Causal multi-head self-attention with RoPE on 8 Trainium2 NeuronCores.

Problem: b=4, s=2048, d_model=1024, 16 heads, dk=64, causal, RoPE(theta=1e4).

Sharding: 8 cores = (batch b, head-half) pairs. Core c handles batch c//2 and
heads (c%2)*8 .. (c%2)*8+8. Each core computes its 8 heads' Q/K/V projections,
causal attention, and a partial output projection; the host sums the two
partials per batch.

Per-core dataflow (all matmuls fp32r, 1 cycle/row on the PE):
  phase 1  Q^T/K^T = (W_perm^T)^T-matmul with moving x^T   -> [128(2 heads), 2048]
           V       = x^T^T-matmul with moving Wv^T         -> [tok, 8*65] ([V|1] interleave)
  phase 2  RoPE in place on Q^T/K^T: pair-swap via SBUF->SBUF DMA + 3 elementwise
  phase 3  per head-pair g, q-chunk c (512): for k-tile kt<=4c+3:
              scores^T[128k, 512q] x2 heads (row-packed K=64 matmuls)
              P = exp(S/8) (one ACT op over both heads), diag-mask multiply
              out^T[65,512] += [V|1]^T @ P  (accumulating PSUM, sums ride row 64)
           staging -> DRAM bounce outT[8, 65, 2048]
  phase 4  reload outT, per-(head,tok) normalize by bcast 1/sums, Wo projection
           y_partial[2048, 1024] -> DRAM

RoPE trick: head-dim pre-permuted (on host, inside W) to [evens; odds] blocks so
rotation = Q*cos + swap32(Q)*sin with pure 32-partition block swaps.
"""
import sys
import numpy as np

for _p in ('/root/.axon_site/_ro/trn_rl_repo', '/opt/trn_rl_repo'):
    if _p not in sys.path:
        sys.path.append(_p)

import concourse.bass as bass
import concourse.tile as tile
from concourse import bacc, mybir
from concourse.bass_utils import run_bass_kernel_spmd

F32 = mybir.dt.float32
F32R = mybir.dt.float32r
EXP = mybir.ActivationFunctionType.Exp

B, S, D = 4, 2048, 1024
NH, DK = 16, 64
NHC = 8            # heads per core
HD = NHC * DK      # 512
NG = 4             # head-pairs (groups) per core
NC_CHUNK = 512     # q-chunk
N_CHUNKS = S // NC_CHUNK       # 4
N_KT = S // 128                # 16
KSUB = D // 128                # 8
THETA = 10000.0

_CACHED = {}


def _build():
    nc = bacc.Bacc('TRN2', target_bir_lowering=False, debug=False, num_devices=8)
    xT = nc.dram_tensor('xT', [D, S], F32R, kind='ExternalInput').ap()
    wqT = nc.dram_tensor('wqT', [D, HD], F32R, kind='ExternalInput').ap()
    wkT = nc.dram_tensor('wkT', [D, HD], F32R, kind='ExternalInput').ap()
    wvT = nc.dram_tensor('wvT', [D, HD], F32R, kind='ExternalInput').ap()
    woT = nc.dram_tensor('woT', [HD, D], F32R, kind='ExternalInput').ap()
    cosd = nc.dram_tensor('cosd', [128, S], F32, kind='ExternalInput').ap()
    sind = nc.dram_tensor('sind', [128, S], F32, kind='ExternalInput').ap()
    maskd = nc.dram_tensor('maskd', [4, 128, 2 * NC_CHUNK], F32, kind='ExternalInput').ap()
    y = nc.dram_tensor('y', [S, D], F32, kind='ExternalOutput').ap()
    outT = nc.dram_tensor('outT', [NHC, DK + 1, S], F32).ap()   # DRAM bounce
    sums_r = nc.dram_tensor('sums_r', [NHC, S], F32).ap()       # raw softmax sums
    recip_d = nc.dram_tensor('recip_d', [NHC, S], F32).ap()     # 1/sums

    with tile.TileContext(nc) as tc:
        with tc.tile_pool(name='persist', bufs=1) as persist:
            q_sb = persist.tile([128, NG, S], F32R, tag='q_sb')
            k_sb = persist.tile([128, NG, S], F32R, tag='k_sb')
            v_sb = persist.tile([128, N_KT, NHC * (DK + 1)], F32R, tag='v_sb')
            v4 = v_sb.rearrange('p t (h m) -> p t h m', h=NHC)

            # ---------------- phase 1: QKV projections ----------------
            with tc.tile_pool(name='xw', bufs=1) as xw, \
                 tc.tile_pool(name='wpool', bufs=12) as wpool, \
                 tc.tile_pool(name='ps1', bufs=3, space='PSUM') as ps1:
                x_sb = xw.tile([128, KSUB, S], F32R, tag='x_sb')
                for s in range(KSUB):
                    nc.sync.dma_start(x_sb[:, s], xT[128 * s:128 * (s + 1), :])

                # Q^T / K^T: [128 hd (2 heads), tok]
                for name, w_ap, dst in (('q', wqT, q_sb), ('k', wkT, k_sb)):
                    w_tiles = []
                    for s in range(KSUB):
                        wt = wpool.tile([128, HD], F32R, tag='w')
                        nc.sync.dma_start(wt[:], w_ap[128 * s:128 * (s + 1), :])
                        w_tiles.append(wt)
                    for g in range(NG):
                        for c in range(N_CHUNKS):
                            ps = ps1.tile([128, NC_CHUNK], F32, tag='ps1')
                            for s in range(KSUB):
                                nc.tensor.matmul(
                                    ps[:], w_tiles[s][:, 128 * g:128 * (g + 1)],
                                    x_sb[:, s, NC_CHUNK * c:NC_CHUNK * (c + 1)],
                                    start=(s == 0), stop=(s == KSUB - 1))
                            nc.scalar.copy(
                                dst[:, g, NC_CHUNK * c:NC_CHUNK * (c + 1)], ps[:])

                # V natural: [tok, hd] with the [V|1] interleave
                w_tiles = []
                for s in range(KSUB):
                    wt = wpool.tile([128, HD], F32R, tag='w')
                    nc.sync.dma_start(wt[:], wvT[128 * s:128 * (s + 1), :])
                    w_tiles.append(wt)
                ones_sb = persist.tile([128, NHC], F32, tag='ones')
                nc.vector.memset(ones_sb[:], 1.0)
                for t in range(N_KT):
                    nc.vector.tensor_copy(v4[:, t, :, DK], ones_sb[:])
                for t in range(N_KT):
                    ps = ps1.tile([128, NC_CHUNK], F32, tag='ps1')
                    for s in range(KSUB):
                        nc.tensor.matmul(
                            ps[:], x_sb[:, s, 128 * t:128 * (t + 1)], w_tiles[s][:],
                            start=(s == 0), stop=(s == KSUB - 1))
                    nc.vector.tensor_copy(
                        v4[:, t, :, 0:DK],
                        ps.rearrange('p (h m) -> p h m', h=NHC))

            # ---------- phases 2+3: RoPE interleaved with attention ----------
            with tc.tile_pool(name='tables', bufs=1) as tables, \
                 tc.tile_pool(name='masks', bufs=1) as maskp, \
                 tc.tile_pool(name='pp', bufs=4) as pp, \
                 tc.tile_pool(name='stage', bufs=3) as stage, \
                 tc.tile_pool(name='ropetmp', bufs=1) as ropetmp, \
                 tc.tile_pool(name='scps', bufs=2, space='PSUM') as scps, \
                 tc.tile_pool(name='outps', bufs=4, space='PSUM') as outps:
                cos_sb = tables.tile([128, S], F32, tag='cos_sb')
                nc.sync.dma_start(cos_sb[:], cosd)
                sin_sb = tables.tile([128, S], F32, tag='sin_sb')
                nc.sync.dma_start(sin_sb[:], sind)
                mask_sb = maskp.tile([128, 4, 2 * NC_CHUNK], F32, tag='mask_sb')
                nc.sync.dma_start(
                    mask_sb[:], bass.AP(
                        tensor=maskd.tensor, offset=maskd.offset,
                        ap=[[2 * NC_CHUNK, 128], [128 * 2 * NC_CHUNK, 4],
                            [1, 2 * NC_CHUNK]]))
                n_mask = 0
                for g in range(NG):
                    # RoPE in place on Q^T[g], K^T[g] (full 2048-wide chunks)
                    for t_sb in (q_sb, k_sb):
                        ch = t_sb[:, g, :]
                        sw = ropetmp.tile([128, S], F32R, tag='swap')
                        for blk in range(4):
                            src = (blk // 2) * 64 + (1 - blk % 2) * 32
                            nc.sync.dma_start(sw[32 * blk:32 * (blk + 1), :],
                                              ch[src:src + 32, :])
                        t1 = ropetmp.tile([128, S], F32, tag='t1')
                        nc.vector.tensor_mul(t1[:], ch, cos_sb[:])
                        t2 = ropetmp.tile([128, S], F32, tag='t2')
                        nc.gpsimd.tensor_tensor(t2[:], sw[:], sin_sb[:],
                                                mybir.AluOpType.mult)
                        nc.vector.tensor_add(ch, t1[:], t2[:])

                    hA, hB = 2 * g, 2 * g + 1
                    for c in range(N_CHUNKS):
                        cs = slice(NC_CHUNK * c, NC_CHUNK * (c + 1))
                        oA = outps.tile([DK + 1, NC_CHUNK], F32, tag='outps')
                        oB = outps.tile([DK + 1, NC_CHUNK], F32, tag='outps')
                        n_kt = 4 * (c + 1)
                        for kt in range(n_kt):
                            ks = slice(128 * kt, 128 * (kt + 1))
                            sc = scps.tile([128, 2 * NC_CHUNK], F32, tag='sc')
                            nc.tensor.matmul(sc[:, 0:NC_CHUNK],
                                             k_sb[0:64, g, ks], q_sb[0:64, g, cs],
                                             start=True, stop=True)
                            nc.tensor.matmul(sc[:, NC_CHUNK:],
                                             k_sb[64:128, g, ks], q_sb[64:128, g, cs],
                                             start=True, stop=True)
                            p = pp.tile([128, 2 * NC_CHUNK], F32R, tag='p')
                            nc.scalar.activation(out=p[:], in_=sc[:], func=EXP,
                                                 scale=1.0 / np.sqrt(DK))
                            if kt >= 4 * c:          # diagonal: mask
                                j = kt - 4 * c
                                eng = nc.vector if n_mask % 2 == 0 else nc.gpsimd
                                eng.tensor_tensor(p[:], p[:], mask_sb[:, j],
                                                  mybir.AluOpType.mult)
                                n_mask += 1
                            nc.tensor.matmul(oA[:], v_sb[:, kt, 65 * hA:65 * (hA + 1)],
                                             p[:, 0:NC_CHUNK],
                                             start=(kt == 0), stop=(kt == n_kt - 1))
                            nc.tensor.matmul(oB[:], v_sb[:, kt, 65 * hB:65 * (hB + 1)],
                                             p[:, NC_CHUNK:],
                                             start=(kt == 0), stop=(kt == n_kt - 1))
                        st = stage.tile([DK + 1, 2 * NC_CHUNK], F32, tag='stage')
                        nc.vector.tensor_copy(st[:, 0:NC_CHUNK], oA[:])
                        nc.vector.tensor_copy(st[:, NC_CHUNK:], oB[:])
                        nc.sync.dma_start(outT[hA, :, cs], st[:, 0:NC_CHUNK])
                        nc.sync.dma_start(outT[hB, :, cs], st[:, NC_CHUNK:])
                        nc.sync.dma_start(sums_r[hA, cs], st[64:65, 0:NC_CHUNK])
                        nc.sync.dma_start(sums_r[hB, cs], st[64:65, NC_CHUNK:])

                # reciprocal of all 8x2048 sums in one dense [128,128] tile
                rcp = stage.tile([128, 128], F32, tag='rcp')
                nc.sync.dma_start(rcp[:], bass.AP(
                    tensor=sums_r.tensor, offset=sums_r.offset,
                    ap=[[128, 128], [1, 128]]))
                nc.vector.reciprocal(rcp[:], rcp[:])
                nc.sync.dma_start(bass.AP(
                    tensor=recip_d.tensor, offset=recip_d.offset,
                    ap=[[128, 128], [1, 128]]), rcp[:])

            # ---------------- phase 4: normalize + output projection ----------------
            with tc.tile_pool(name='wo', bufs=1) as wo, \
                 tc.tile_pool(name='lhs', bufs=1) as lhs, \
                 tc.tile_pool(name='otrc', bufs=2) as otrc, \
                 tc.tile_pool(name='ytmp', bufs=4) as ytmp, \
                 tc.tile_pool(name='ps4', bufs=4, space='PSUM') as ps4:
                wo_sb = wo.tile([128, NG, D], F32R, tag='wo_sb')
                for g in range(NG):
                    nc.sync.dma_start(wo_sb[:, g], woT[128 * g:128 * (g + 1), :])
                lhs_sb = lhs.tile([128, NG, S], F32R, tag='lhs_sb')
                for g in range(NG):
                    hA, hB = 2 * g, 2 * g + 1
                    ot = otrc.tile([128, S], F32, tag='ot')
                    nc.sync.dma_start(ot[0:64, :], outT[hA, 0:DK, :])
                    nc.sync.dma_start(ot[64:128, :], outT[hB, 0:DK, :])
                    rc = otrc.tile([128, S], F32, tag='rc')
                    for half, h in ((0, hA), (1, hB)):
                        nc.sync.dma_start(
                            rc[64 * half:64 * (half + 1), :],
                            bass.AP(tensor=recip_d.tensor,
                                    offset=recip_d.offset + h * S,
                                    ap=[[0, 64], [1, S]]))
                    nc.vector.tensor_mul(lhs_sb[:, g], ot[:], rc[:])

                for t in range(N_KT):
                    ts_ = slice(128 * t, 128 * (t + 1))
                    for half in range(2):
                        ps = ps4.tile([128, 512], F32, tag='ps4')
                        for g in range(NG):
                            nc.tensor.matmul(ps[:], lhs_sb[:, g, ts_],
                                             wo_sb[:, g, 512 * half:512 * (half + 1)],
                                             start=(g == 0), stop=(g == NG - 1))
                        yt = ytmp.tile([128, 512], F32, tag='yt')
                        nc.scalar.copy(yt[:], ps[:])
                        nc.sync.dma_start(y[ts_, 512 * half:512 * (half + 1)], yt[:])
    nc.compile()
    return nc


def _host_inputs(x, Wq, Wk, Wv, Wo, token_positions):
    """Per-core input maps (host-side sharding / layout only)."""
    perm = np.empty(DK, np.int64)
    perm[0:32] = np.arange(0, DK, 2)
    perm[32:64] = np.arange(1, DK, 2)

    inv_freq = 1.0 / (THETA ** (np.arange(0, DK, 2, dtype=np.float64) / DK))  # [32]
    ang = token_positions.astype(np.float64)[None, :] * inv_freq[:, None]     # [32, S]
    cos32 = np.cos(ang).astype(np.float32)
    sin32 = np.sin(ang).astype(np.float32)
    cos128 = np.tile(cos32, (4, 1))
    sin128 = np.concatenate([-sin32, sin32, -sin32, sin32], axis=0)

    qc = np.arange(NC_CHUNK)[None, :]
    kr = np.arange(128)[:, None]
    masks = np.stack([(qc >= 128 * j + kr).astype(np.float32) for j in range(4)])
    masks = np.concatenate([masks, masks], axis=2)  # [4, 128, 1024]

    in_maps = []
    for core in range(8):
        b = core // 2
        h0 = (core % 2) * NHC
        cols = slice(h0 * DK, (h0 + NHC) * DK)
        wq_s = Wq[cols, :].reshape(NHC, DK, D)[:, perm, :].reshape(HD, D)
        wk_s = Wk[cols, :].reshape(NHC, DK, D)[:, perm, :].reshape(HD, D)
        in_maps.append({
            'xT': np.ascontiguousarray(x[b].T),
            'wqT': np.ascontiguousarray(wq_s.T),
            'wkT': np.ascontiguousarray(wk_s.T),
            'wvT': np.ascontiguousarray(Wv[cols, :].T),
            'woT': np.ascontiguousarray(Wo[:, cols].T),
            'cosd': cos128, 'sind': sin128, 'maskd': masks,
        })
    return in_maps


def kernel(x, Wq, Wk, Wv, Wo, token_positions, _results_hook=None):
    if 'nc' not in _CACHED:
        _CACHED['nc'] = _build()
    nc = _CACHED['nc']
    in_maps = _host_inputs(np.asarray(x), np.asarray(Wq), np.asarray(Wk),
                           np.asarray(Wv), np.asarray(Wo),
                           np.asarray(token_positions))
    res = run_bass_kernel_spmd(nc, in_maps, list(range(8)),
                               **(_results_hook or {}))
    if _results_hook is not None:
        _CACHED['last'] = res
    out = np.empty((B, S, D), np.float32)
    for b in range(B):
        out[b] = res.results[2 * b]['y'] + res.results[2 * b + 1]['y']
    return out


# revision 15
# speedup vs baseline: 1.2131x; 1.0801x over previous
"""Causal multi-head self-attention with RoPE on 8 Trainium2 NeuronCores.

Problem: b=4, s=2048, d_model=1024, 16 heads, dk=64, causal, RoPE(theta=1e4).

Sharding: 8 cores = (batch b, head-half) pairs. Core c handles batch c//2 and
heads (c%2)*8 .. (c%2)*8+8. Each core computes its 8 heads' Q/K/V projections,
causal attention, and a partial output projection; the host sums the two
partials per batch.

Per-core dataflow (all matmuls fp32r, 1 cycle/row on the PE):
  phase 1  Q^T/K^T = (W_perm^T)^T-matmul with moving x^T   -> [128(2 heads), 2048]
           V       = x^T^T-matmul with moving Wv^T         -> [tok, 8*65] ([V|1] interleave)
  phase 2  RoPE in place on Q^T/K^T: pair-swap via SBUF->SBUF DMA + 3 elementwise
  phase 3  per head-pair g, q-chunk c (512): for k-tile kt<=4c+3:
              scores^T[128k, 512q] x2 heads (row-packed K=64 matmuls)
              P = exp(S/8) (one ACT op over both heads), diag-mask multiply
              out^T[65,512] += [V|1]^T @ P  (accumulating PSUM, sums ride row 64)
           staging -> DRAM bounce outT[8, 65, 2048]
  phase 4  reload outT, per-(head,tok) normalize by bcast 1/sums, Wo projection
           y_partial[2048, 1024] -> DRAM

RoPE trick: head-dim pre-permuted (on host, inside W) to [evens; odds] blocks so
rotation = Q*cos + swap32(Q)*sin with pure 32-partition block swaps.
"""
import sys
import numpy as np

for _p in ('/root/.axon_site/_ro/trn_rl_repo', '/opt/trn_rl_repo'):
    if _p not in sys.path:
        sys.path.append(_p)

import concourse.bass as bass
import concourse.tile as tile
from concourse import bacc, mybir
from concourse.bass_utils import run_bass_kernel_spmd

F32 = mybir.dt.float32
F32R = mybir.dt.float32r
EXP = mybir.ActivationFunctionType.Exp

B, S, D = 4, 2048, 1024
NH, DK = 16, 64
NHC = 8            # heads per core
HD = NHC * DK      # 512
NG = 4             # head-pairs (groups) per core
NC_CHUNK = 512     # q-chunk
N_CHUNKS = S // NC_CHUNK       # 4
N_KT = S // 128                # 16
KSUB = D // 128                # 8
THETA = 10000.0

_CACHED = {}


def _build():
    nc = bacc.Bacc('TRN2', target_bir_lowering=False, debug=False, num_devices=8)
    xT = nc.dram_tensor('xT', [D, S], F32R, kind='ExternalInput').ap()
    wqT = nc.dram_tensor('wqT', [D, HD], F32R, kind='ExternalInput').ap()
    wkT = nc.dram_tensor('wkT', [D, HD], F32R, kind='ExternalInput').ap()
    wvT = nc.dram_tensor('wvT', [D, HD], F32R, kind='ExternalInput').ap()
    woT = nc.dram_tensor('woT', [HD, D], F32R, kind='ExternalInput').ap()
    cosd = nc.dram_tensor('cosd', [128, S], F32, kind='ExternalInput').ap()
    sind = nc.dram_tensor('sind', [128, S], F32, kind='ExternalInput').ap()
    maskd = nc.dram_tensor('maskd', [128, 128], F32, kind='ExternalInput').ap()
    y = nc.dram_tensor('y', [S, D], F32, kind='ExternalOutput').ap()
    outT = nc.dram_tensor('outT', [NHC, DK + 1, S], F32).ap()   # DRAM bounce
    sums_r = nc.dram_tensor('sums_r', [NHC, S], F32).ap()       # raw softmax sums
    recip_d = nc.dram_tensor('recip_d', [NHC, S], F32).ap()     # 1/sums

    with tile.TileContext(nc) as tc:
        with tc.tile_pool(name='persist', bufs=1) as persist:
            q_sb = persist.tile([128, NG, S], F32R, tag='q_sb')
            k_sb = persist.tile([128, NG, S], F32R, tag='k_sb')
            v_sb = persist.tile([128, N_KT, NHC * (DK + 1)], F32R, tag='v_sb')
            v4 = v_sb.rearrange('p t (h m) -> p t h m', h=NHC)

            # ---------------- phase 1: QKV projections ----------------
            with tc.tile_pool(name='xw', bufs=1) as xw, \
                 tc.tile_pool(name='wpool', bufs=12) as wpool, \
                 tc.tile_pool(name='ps1', bufs=3, space='PSUM') as ps1:
                x_sb = xw.tile([128, KSUB, S], F32R, tag='x_sb')
                for s in range(KSUB):
                    nc.sync.dma_start(x_sb[:, s], xT[128 * s:128 * (s + 1), :])

                # Q^T / K^T: [128 hd (2 heads), tok]
                for name, w_ap, dst in (('q', wqT, q_sb), ('k', wkT, k_sb)):
                    w_tiles = []
                    for s in range(KSUB):
                        wt = wpool.tile([128, HD], F32R, tag='w')
                        nc.sync.dma_start(wt[:], w_ap[128 * s:128 * (s + 1), :])
                        w_tiles.append(wt)
                    for g in range(NG):
                        for c in range(N_CHUNKS):
                            ps = ps1.tile([128, NC_CHUNK], F32, tag='ps1')
                            for s in range(KSUB):
                                nc.tensor.matmul(
                                    ps[:], w_tiles[s][:, 128 * g:128 * (g + 1)],
                                    x_sb[:, s, NC_CHUNK * c:NC_CHUNK * (c + 1)],
                                    start=(s == 0), stop=(s == KSUB - 1))
                            nc.vector.tensor_copy(
                                dst[:, g, NC_CHUNK * c:NC_CHUNK * (c + 1)], ps[:])

                # V natural: [tok, hd] with the [V|1] interleave
                w_tiles = []
                for s in range(KSUB):
                    wt = wpool.tile([128, HD], F32R, tag='w')
                    nc.sync.dma_start(wt[:], wvT[128 * s:128 * (s + 1), :])
                    w_tiles.append(wt)
                ones_sb = persist.tile([128, NHC], F32, tag='ones')
                nc.vector.memset(ones_sb[:], 1.0)
                for t in range(N_KT):
                    nc.vector.tensor_copy(v4[:, t, :, DK], ones_sb[:])
                for t in range(N_KT):
                    ps = ps1.tile([128, NC_CHUNK], F32, tag='ps1')
                    for s in range(KSUB):
                        nc.tensor.matmul(
                            ps[:], x_sb[:, s, 128 * t:128 * (t + 1)], w_tiles[s][:],
                            start=(s == 0), stop=(s == KSUB - 1))
                    nc.vector.tensor_copy(
                        v4[:, t, :, 0:DK],
                        ps.rearrange('p (h m) -> p h m', h=NHC))

            # ---------- phases 2+3: RoPE interleaved with attention ----------
            with tc.tile_pool(name='tables', bufs=1) as tables, \
                 tc.tile_pool(name='masks', bufs=1) as maskp, \
                 tc.tile_pool(name='pp', bufs=4) as pp, \
                 tc.tile_pool(name='stage', bufs=3) as stage, \
                 tc.tile_pool(name='ropetmp', bufs=1) as ropetmp, \
                 tc.tile_pool(name='scps', bufs=2, space='PSUM') as scps, \
                 tc.tile_pool(name='outps', bufs=4, space='PSUM') as outps:
                cos_sb = tables.tile([128, S], F32, tag='cos_sb')
                nc.sync.dma_start(cos_sb[:], cosd)
                sin_sb = tables.tile([128, S], F32, tag='sin_sb')
                nc.sync.dma_start(sin_sb[:], sind)
                tri_sb = maskp.tile([128, 128], F32, tag='tri_sb')
                nc.sync.dma_start(tri_sb[:], maskd)
                for g in range(NG):
                    # RoPE in place on Q^T[g], K^T[g] (full 2048-wide chunks)
                    for t_sb in (q_sb, k_sb):
                        ch = t_sb[:, g, :]
                        sw = ropetmp.tile([128, S], F32R, tag='swap')
                        for blk in range(4):
                            src = (blk // 2) * 64 + (1 - blk % 2) * 32
                            nc.sync.dma_start(sw[32 * blk:32 * (blk + 1), :],
                                              ch[src:src + 32, :])
                        t1 = ropetmp.tile([128, S], F32, tag='t1')
                        nc.vector.tensor_mul(t1[:], ch, cos_sb[:])
                        t2 = ropetmp.tile([128, S], F32, tag='t2')
                        nc.gpsimd.tensor_tensor(t2[:], sw[:], sin_sb[:],
                                                mybir.AluOpType.mult)
                        nc.vector.tensor_add(ch, t1[:], t2[:])

                    hA, hB = 2 * g, 2 * g + 1
                    for c in range(N_CHUNKS):
                        cs = slice(NC_CHUNK * c, NC_CHUNK * (c + 1))
                        oA = outps.tile([DK + 1, NC_CHUNK], F32, tag='outps')
                        oB = outps.tile([DK + 1, NC_CHUNK], F32, tag='outps')
                        n_kt = 4 * (c + 1)
                        for kt in range(n_kt):
                            ks = slice(128 * kt, 128 * (kt + 1))
                            j = kt - 4 * c          # >=0 on the diagonal
                            v0 = max(j, 0) * 128    # first valid q column
                            sA = slice(v0, NC_CHUNK)
                            sB = slice(NC_CHUNK + v0, 2 * NC_CHUNK)
                            qv = q_sb[:, g, NC_CHUNK * c + v0:NC_CHUNK * (c + 1)]
                            sc = scps.tile([128, 2 * NC_CHUNK], F32, tag='sc')
                            nc.tensor.matmul(sc[:, sA], k_sb[0:64, g, ks], qv[0:64],
                                             start=True, stop=True)
                            nc.tensor.matmul(sc[:, sB], k_sb[64:128, g, ks], qv[64:128],
                                             start=True, stop=True)
                            p = pp.tile([128, 2 * NC_CHUNK], F32R, tag='p')
                            if v0 == 0:
                                nc.scalar.activation(out=p[:], in_=sc[:], func=EXP,
                                                     scale=1.0 / np.sqrt(DK))
                            else:
                                nc.scalar.activation(out=p[:, sA], in_=sc[:, sA],
                                                     func=EXP, scale=1.0 / np.sqrt(DK))
                                nc.scalar.activation(out=p[:, sB], in_=sc[:, sB],
                                                     func=EXP, scale=1.0 / np.sqrt(DK))
                            if j >= 0:               # triangle on the diag sub-block
                                dA = slice(v0, v0 + 128)
                                dB = slice(NC_CHUNK + v0, NC_CHUNK + v0 + 128)
                                nc.vector.tensor_tensor(p[:, dA], p[:, dA], tri_sb[:],
                                                        mybir.AluOpType.mult)
                                nc.gpsimd.tensor_tensor(p[:, dB], p[:, dB], tri_sb[:],
                                                        mybir.AluOpType.mult)
                            nc.tensor.matmul(oA[:, sA], v_sb[:, kt, 65 * hA:65 * (hA + 1)],
                                             p[:, sA],
                                             start=(kt == 0), stop=(kt == n_kt - 1))
                            nc.tensor.matmul(oB[:, sA], v_sb[:, kt, 65 * hB:65 * (hB + 1)],
                                             p[:, sB],
                                             start=(kt == 0), stop=(kt == n_kt - 1))
                        st = stage.tile([DK + 1, 2 * NC_CHUNK], F32, tag='stage')
                        nc.vector.tensor_copy(st[:, 0:NC_CHUNK], oA[:])
                        nc.vector.tensor_copy(st[:, NC_CHUNK:], oB[:])
                        nc.sync.dma_start(outT[hA, :, cs], st[:, 0:NC_CHUNK])
                        nc.sync.dma_start(outT[hB, :, cs], st[:, NC_CHUNK:])
                        nc.sync.dma_start(sums_r[hA, cs], st[64:65, 0:NC_CHUNK])
                        nc.sync.dma_start(sums_r[hB, cs], st[64:65, NC_CHUNK:])

                # reciprocal of all 8x2048 sums in one dense [128,128] tile
                rcp = stage.tile([128, 128], F32, tag='rcp')
                nc.sync.dma_start(rcp[:], bass.AP(
                    tensor=sums_r.tensor, offset=sums_r.offset,
                    ap=[[128, 128], [1, 128]]))
                nc.vector.reciprocal(rcp[:], rcp[:])
                nc.sync.dma_start(bass.AP(
                    tensor=recip_d.tensor, offset=recip_d.offset,
                    ap=[[128, 128], [1, 128]]), rcp[:])

            # ---------------- phase 4: normalize + output projection ----------------
            with tc.tile_pool(name='wo', bufs=1) as wo, \
                 tc.tile_pool(name='lhs', bufs=1) as lhs, \
                 tc.tile_pool(name='otrc', bufs=2) as otrc, \
                 tc.tile_pool(name='ytmp', bufs=4) as ytmp, \
                 tc.tile_pool(name='ps4', bufs=4, space='PSUM') as ps4:
                wo_sb = wo.tile([128, NG, D], F32R, tag='wo_sb')
                for g in range(NG):
                    nc.sync.dma_start(wo_sb[:, g], woT[128 * g:128 * (g + 1), :])
                lhs_sb = lhs.tile([128, NG, S], F32R, tag='lhs_sb')
                for g in range(NG):
                    hA, hB = 2 * g, 2 * g + 1
                    ot = otrc.tile([128, S], F32, tag='ot')
                    nc.sync.dma_start(ot[0:64, :], outT[hA, 0:DK, :])
                    nc.sync.dma_start(ot[64:128, :], outT[hB, 0:DK, :])
                    rc = otrc.tile([128, S], F32, tag='rc')
                    for half, h in ((0, hA), (1, hB)):
                        nc.sync.dma_start(
                            rc[64 * half:64 * (half + 1), :],
                            bass.AP(tensor=recip_d.tensor,
                                    offset=recip_d.offset + h * S,
                                    ap=[[0, 64], [1, S]]))
                    nc.vector.tensor_mul(lhs_sb[:, g], ot[:], rc[:])

                for t in range(N_KT):
                    ts_ = slice(128 * t, 128 * (t + 1))
                    for half in range(2):
                        ps = ps4.tile([128, 512], F32, tag='ps4')
                        for g in range(NG):
                            nc.tensor.matmul(ps[:], lhs_sb[:, g, ts_],
                                             wo_sb[:, g, 512 * half:512 * (half + 1)],
                                             start=(g == 0), stop=(g == NG - 1))
                        yt = ytmp.tile([128, 512], F32, tag='yt')
                        nc.scalar.copy(yt[:], ps[:])
                        nc.sync.dma_start(y[ts_, 512 * half:512 * (half + 1)], yt[:])
    nc.compile()
    return nc


def _host_inputs(x, Wq, Wk, Wv, Wo, token_positions):
    """Per-core input maps (host-side sharding / layout only)."""
    perm = np.empty(DK, np.int64)
    perm[0:32] = np.arange(0, DK, 2)
    perm[32:64] = np.arange(1, DK, 2)

    inv_freq = 1.0 / (THETA ** (np.arange(0, DK, 2, dtype=np.float64) / DK))  # [32]
    ang = token_positions.astype(np.float64)[None, :] * inv_freq[:, None]     # [32, S]
    cos32 = np.cos(ang).astype(np.float32)
    sin32 = np.sin(ang).astype(np.float32)
    cos128 = np.tile(cos32, (4, 1))
    sin128 = np.concatenate([-sin32, sin32, -sin32, sin32], axis=0)

    tri = (np.arange(128)[None, :] >= np.arange(128)[:, None]).astype(np.float32)

    in_maps = []
    for core in range(8):
        b = core // 2
        h0 = (core % 2) * NHC
        cols = slice(h0 * DK, (h0 + NHC) * DK)
        wq_s = Wq[cols, :].reshape(NHC, DK, D)[:, perm, :].reshape(HD, D)
        wk_s = Wk[cols, :].reshape(NHC, DK, D)[:, perm, :].reshape(HD, D)
        in_maps.append({
            'xT': np.ascontiguousarray(x[b].T),
            'wqT': np.ascontiguousarray(wq_s.T),
            'wkT': np.ascontiguousarray(wk_s.T),
            'wvT': np.ascontiguousarray(Wv[cols, :].T),
            'woT': np.ascontiguousarray(Wo[:, cols].T),
            'cosd': cos128, 'sind': sin128, 'maskd': tri,
        })
    return in_maps


def kernel(x, Wq, Wk, Wv, Wo, token_positions, _results_hook=None):
    if 'nc' not in _CACHED:
        _CACHED['nc'] = _build()
    nc = _CACHED['nc']
    in_maps = _host_inputs(np.asarray(x), np.asarray(Wq), np.asarray(Wk),
                           np.asarray(Wv), np.asarray(Wo),
                           np.asarray(token_positions))
    res = run_bass_kernel_spmd(nc, in_maps, list(range(8)),
                               **(_results_hook or {}))
    if _results_hook is not None:
        _CACHED['last'] = res
    out = np.empty((B, S, D), np.float32)
    for b in range(B):
        out[b] = res.results[2 * b]['y'] + res.results[2 * b + 1]['y']
    return out
